# revision 4
# baseline (speedup 1.0000x reference)
"""Multi-head causal attention (B=4, S=2048, D=1024, H=16) on 8 cores, v4.

Sharding: core c -> batch b = c//2, head-group g = c%2 (8 heads each).

HW-calibrated design (this session's axon TRN2 microbenchmarks):
  - bf16 matmul K=128 [*,512] = 204ns (1 cyc/col); K=64 standalone is
    2 cyc/col, BUT two K=64 streams at PE row offsets 0/64 writing
    DIFFERENT PSUM tiles run concurrently (91.6ns each) -> scores are
    issued head-paired (even head rows 0:63, odd head rows 64:127) and
    AV is issued lo/hi-paired per head.
  - Same-PSUM cross-row-position accumulation crashes the compiler, so
    paired AV halves land in separate [65,512] PSUM tiles merged by DVE
    (tensor_tensor with one PSUM + one SBUF operand works; two PSUM
    operands crash).
  - ACT exp with rotating dst tiles = 0.70ns/col at 512 wide (as good
    as 1024 wide) -> 512-wide q-blocks, PSUM fits in 8 banks:
    pp A/B (2) + psO lo/hi x A/B (4) + psY (2).
  - W_O for q-block qb-1 is interleaved into qb's attention so the PE
    fills ACT-bound gaps; ACT does only Exp (table reloads avoided).
"""

import numpy as np

import concourse.bass as bass
import concourse.tile as tile
import concourse.mybir as mybir
from concourse import bacc
from concourse.bass_utils import run_bass_kernel_spmd

B, S, D, H, HD = 4, 2048, 1024, 16, 64
NH = 8            # heads per core
NP = NH // 2      # head pairs per core
QB = 512          # s-chunk and q-block size
NQB = S // QB     # 4
NKT = D // 128    # 8 contraction tiles over D
NST = S // 128    # 16 t-chunks
NDG = QB // 128   # 4 diag sub-chunks per q-block
CW = NH * HD      # 512 concat width per core

F32 = mybir.dt.float32
BF16 = mybir.dt.bfloat16
F16 = mybir.dt.float16
AF = mybir.ActivationFunctionType

N_CORES = 8

_cache = {}


def build_nc(repeats=1, phases="full", hw_loop=False):
    nc = bacc.Bacc("TRN2", target_bir_lowering=False, debug=False,
                   num_devices=N_CORES)
    x_ck = nc.dram_tensor("x_ck", [NQB, 128, NKT, QB], F16,
                          kind="ExternalInput").ap()
    wq = nc.dram_tensor("wq", [128, NKT, CW], F16, kind="ExternalInput").ap()
    wk = nc.dram_tensor("wk", [128, NKT, CW], F16, kind="ExternalInput").ap()
    wv = nc.dram_tensor("wv", [128, NKT, CW], F16, kind="ExternalInput").ap()
    wot = nc.dram_tensor("wot", [128, CW // 128, D], F16,
                         kind="ExternalInput").ap()
    masks = nc.dram_tensor("masks", [128, 128], F16,
                           kind="ExternalInput").ap()
    ones = nc.dram_tensor("ones", [128, NST * NH], F16,
                          kind="ExternalInput").ap()
    y = nc.dram_tensor("y", [S, D], F16, kind="ExternalOutput").ap()

    with tile.TileContext(nc) as tc:
        if hw_loop:
            with tc.For_i(0, repeats, 1):
                _build(tc, x_ck, wq, wk, wv, wot, masks, ones, y, phases)
        else:
            for _ in range(repeats):
                _build(tc, x_ck, wq, wk, wv, wot, masks, ones, y, phases)
    nc.compile()
    return nc


def _build(tc, x_ck, wq, wk, wv, wot, masks, ones, y, phases="full"):
    nc = tc.nc
    with (
        tc.tile_pool(name="persist", bufs=1) as persist,
        tc.tile_pool(name="xsp", bufs=4) as xsp,
        tc.tile_pool(name="exp", bufs=4) as exp_,
        tc.tile_pool(name="nrm", bufs=3) as nrm,
        tc.tile_pool(name="ysp", bufs=4) as ysp,
    ):
        qt_sb = persist.tile([128, NP, S], F16)
        kt_sb = persist.tile([128, NP, S], F16)
        v_sb = persist.tile([128, NST, NH, HD + 1], F16)
        conc_sb = persist.tile([128, NP, S], F16)
        tri_sb = persist.tile([128, 128], F16)
        wq_sb = persist.tile([128, NKT, CW], F16)
        wk_sb = persist.tile([128, NKT, CW], F16)
        wv_sb = persist.tile([128, NKT, CW], F16)
        wot_sb = persist.tile([128, CW // 128, D], F16)

        nc.gpsimd.dma_start(wq_sb, wq)
        nc.gpsimd.dma_start(wv_sb, wv)
        nc.gpsimd.dma_start(wk_sb, wk)
        nc.gpsimd.dma_start(wot_sb, wot)
        nc.gpsimd.dma_start(tri_sb, masks)
        v_ones = bass.AP(tensor=v_sb.tensor, offset=v_sb.offset + HD,
                         ap=[list(v_sb.ap[0]), [HD + 1, NST * NH], [1, 1]])
        nc.vector.memset(v_ones, 1.0)

        for c in range(NQB):
            # ---- projections for s-chunk c (2 PSUM banks, 12 tiles) ----
            with tc.tile_pool(name=f"pj{c}", bufs=4, space="PSUM") as pjp:
                xs = xsp.tile([128, NKT, QB], F16, tag="xs")
                nc.sync.dma_start(xs, x_ck[c])
                for p in range(NP):
                    ps = pjp.tile([128, QB], F32, tag="pj", name=f"q{c}{p}")
                    for k in range(NKT):
                        nc.tensor.matmul(ps,
                                         wq_sb[:, k, p * 128:(p + 1) * 128],
                                         xs[:, k, :],
                                         start=(k == 0), stop=(k == NKT - 1))
                    nc.vector.tensor_copy(
                        qt_sb[:, p, c * QB:(c + 1) * QB], ps)
                for i in range(4):
                    ps = pjp.tile([128, QB], F32, tag="pj", name=f"v{c}{i}")
                    for k in range(NKT):
                        nc.tensor.matmul(ps,
                                         xs[:, k, i * 128:(i + 1) * 128],
                                         wv_sb[:, k, :],
                                         start=(k == 0), stop=(k == NKT - 1))
                    nc.vector.tensor_copy(
                        v_sb[:, c * 4 + i, :, 0:HD],
                        ps.rearrange("p (h e) -> p h e", h=NH))
                for p in range(NP):
                    ps = pjp.tile([128, QB], F32, tag="pj", name=f"k{c}{p}")
                    for k in range(NKT):
                        nc.tensor.matmul(ps,
                                         wk_sb[:, k, p * 128:(p + 1) * 128],
                                         xs[:, k, :],
                                         start=(k == 0), stop=(k == NKT - 1))
                    nc.vector.tensor_copy(
                        kt_sb[:, p, c * QB:(c + 1) * QB], ps)

            if phases == "proj":
                continue

            # ---- attention for q-block qb = c, + W_O for qb-1 ----------
            qb = c
            ntc = (qb + 1) * NDG  # t-chunks for this q-block
            with (
                tc.tile_pool(name=f"app{qb}", bufs=1, space="PSUM") as app,
                tc.tile_pool(name=f"apo{qb}", bufs=1, space="PSUM") as apo,
                tc.tile_pool(name=f"apy{qb}", bufs=2, space="PSUM") as apy,
            ):
                for hp in range(NP):
                    hA, hB = 2 * hp, 2 * hp + 1
                    pso = {}
                    for nm in ("Alo", "Ahi", "Blo", "Bhi"):
                        pso[nm] = apo.tile([HD + 1, QB], F32, tag=f"o{nm}",
                                           name=f"o{nm}")
                    prev = None
                    for t in range(ntc):
                        diag = t >= qb * NDG
                        q0 = (t - qb * NDG) * 128 if diag else 0
                        ppA = app.tile([128, QB], F32, tag="ppA", name="ppA")
                        ppB = app.tile([128, QB], F32, tag="ppB", name="ppB")
                        ts = slice(t * 128, (t + 1) * 128)
                        qs = slice(qb * QB + q0, (qb + 1) * QB)
                        nc.tensor.matmul(ppA[:, q0:],
                                         kt_sb[0:64, hp, ts],
                                         qt_sb[0:64, hp, qs],
                                         start=True, stop=True)
                        nc.tensor.matmul(ppB[:, q0:],
                                         kt_sb[64:128, hp, ts],
                                         qt_sb[64:128, hp, qs],
                                         start=True, stop=True)
                        exA = exp_.tile([128, QB], F16, tag="exA",
                                        name="exA")
                        exB = exp_.tile([128, QB], F16, tag="exB",
                                        name="exB")
                        nc.scalar.activation(exA[:, q0:], ppA[:, q0:],
                                             AF.Exp, scale=0.125)
                        nc.scalar.activation(exB[:, q0:], ppB[:, q0:],
                                             AF.Exp, scale=0.125)
                        if diag:
                            nc.vector.tensor_mul(exA[:, q0:q0 + 128],
                                                 exA[:, q0:q0 + 128], tri_sb)
                            nc.vector.tensor_mul(exB[:, q0:q0 + 128],
                                                 exB[:, q0:q0 + 128], tri_sb)
                        if prev is not None:
                            _av(nc, pso, v_sb, prev, hA, hB, qb)
                        prev = (t, q0, exA, exB)
                    _av(nc, pso, v_sb, prev, hA, hB, qb)

                    # normalize both heads -> concat
                    for nm, r0 in (("A", 0), ("B", 64)):
                        sm = nrm.tile([HD + 1, QB], F32, tag=f"sm{nm}",
                                      name=f"sm{nm}")
                        nc.vector.tensor_copy(sm, pso[nm + "lo"])
                        sm2 = nrm.tile([HD + 1, QB], F32, tag=f"s2{nm}",
                                       name=f"s2{nm}")
                        nc.vector.tensor_add(sm2, pso[nm + "hi"], sm)
                        rec = nrm.tile([1, QB], F32, tag=f"rc{nm}",
                                       name=f"rc{nm}")
                        nc.vector.reciprocal(rec, sm2[HD:HD + 1, :])
                        rec_b = nrm.tile([64, QB], F32, tag=f"rb{nm}",
                                         name=f"rb{nm}")
                        nc.gpsimd.partition_broadcast(rec_b, rec)
                        nc.vector.tensor_mul(
                            conc_sb[r0:r0 + 64, hp,
                                    qb * QB:(qb + 1) * QB],
                            sm2[0:HD, :], rec_b)

                    # W_O for previous q-block, one row-tile per head-pair
                    if qb > 0:
                        _wo(nc, apy, ysp, conc_sb, wot_sb, y,
                            (qb - 1) * NDG + hp)

        if phases == "proj":
            return
        # tail: W_O for the last q-block
        with tc.tile_pool(name="apyt", bufs=2, space="PSUM") as apy:
            for j in range(NDG):
                _wo(nc, apy, ysp, conc_sb, wot_sb, y, 3 * NDG + j)


def _av(nc, pso, v_sb, prev, hA, hB, qb):
    t, q0, exA, exB = prev
    last = q0 == QB - 128
    start = t == 0
    for nm, ex, h in (("A", exA, hA), ("B", exB, hB)):
        nc.tensor.matmul(pso[nm + "lo"][:, q0:],
                         v_sb[0:64, t, h, :], ex[0:64, q0:],
                         start=start, stop=last)
        nc.tensor.matmul(pso[nm + "hi"][:, q0:],
                         v_sb[64:128, t, h, :], ex[64:128, q0:],
                         start=start, stop=last)


def _wo(nc, apy, ysp, conc_sb, wot_sb, y, tdx):
    ysb = ysp.tile([128, D], F16, tag="ysb", name="ysb")
    for dh in range(2):
        psy = apy.tile([128, 512], F32, tag="psy", name="psy")
        for cc in range(CW // 128):
            nc.tensor.matmul(psy,
                             conc_sb[:, cc, tdx * 128:(tdx + 1) * 128],
                             wot_sb[:, cc, dh * 512:(dh + 1) * 512],
                             start=(cc == 0), stop=(cc == CW // 128 - 1))
        nc.vector.tensor_copy(ysb[:, dh * 512:(dh + 1) * 512], psy)
    nc.sync.dma_start(y[tdx * 128:(tdx + 1) * 128, :], ysb)


def shard_inputs(x, Wq, Wk, Wv, W_O):
    """Build the 8 per-core input maps from full inputs (host, untimed)."""
    f16 = mybir.dt.np(F16)
    masks = (np.arange(128)[:, None] <= np.arange(128)[None, :]).astype(f16)

    def wtile(w):
        # [D, CW] -> [128, NKT, CW] with row d = k*128 + r
        return np.ascontiguousarray(
            w.reshape(NKT, 128, CW).transpose(1, 0, 2)).astype(f16)

    in_maps = []
    for c in range(N_CORES):
        b, g = c // 2, c % 2
        hs = slice(g * NH, (g + 1) * NH)
        xT = np.ascontiguousarray(x[b].T)
        x_ck = np.ascontiguousarray(
            xT.reshape(NKT, 128, NQB, QB).transpose(2, 1, 0, 3)).astype(f16)
        wot = np.ascontiguousarray(W_O[:, g * CW:(g + 1) * CW].T)
        in_maps.append({
            "x_ck": x_ck,
            "wq": wtile(Wq[hs].transpose(1, 0, 2).reshape(D, CW)),
            "wk": wtile(Wk[hs].transpose(1, 0, 2).reshape(D, CW)),
            "wv": wtile(Wv[hs].transpose(1, 0, 2).reshape(D, CW)),
            "wot": np.ascontiguousarray(
                wot.reshape(CW // 128, 128, D).transpose(1, 0, 2)).astype(
                    f16),
            "masks": masks,
            "ones": np.ones((128, NST * NH), f16),
        })
    return in_maps


def kernel(x, Wq, Wk, Wv, W_O):
    x = np.asarray(x, np.float32)
    Wq = np.asarray(Wq, np.float32)
    Wk = np.asarray(Wk, np.float32)
    Wv = np.asarray(Wv, np.float32)
    W_O = np.asarray(W_O, np.float32)

    if "nc" not in _cache:
        _cache["nc"] = build_nc()
    nc = _cache["nc"]

    in_maps = shard_inputs(x, Wq, Wk, Wv, W_O)
    res = run_bass_kernel_spmd(nc, in_maps, core_ids=list(range(N_CORES)))
    _cache["last_results"] = res

    y = np.zeros((B, S, D), np.float32)
    for c in range(N_CORES):
        y[c // 2] += res.results[c]["y"].astype(np.float32)
    return y
